# revision 14
# baseline (speedup 1.0000x reference)
"""MoE (top-K routing, per-expert capacity) Trainium2 kernel.

Strategy: expert parallelism across 8 NeuronCores (E=8, one expert per core).
 - Host: routing top-C selection per expert (tiny: E x T scores), gather of
   dispatched tokens, and fold of the combine weights ("gain") into the
   dispatched activations. gain >= 0 (softmax outputs), so
   gain * (relu(xe@W1)@W2) == relu((gain*xe)@W1)@W2 exactly in math terms.
 - Device (per core): fused 2-layer MLP in a single hand-written Tile kernel:
       hT = relu(W1.T @ xeT)   (F, Ca)   hT kept in SBUF, F in G groups
       y  = hT.T @ W2          (Ca, D)   PSUM-accumulated per group,
                                         DVE-accumulated across groups
   float32r matmuls (full PE stream rate, fp32-class data).
 - Host: per-expert scatter-add of y_e back into the (T, D) output.

Only the active capacity prefix Ca <= C is computed: top-C ordering sorts
valid slots first, so slots >= max_e(n_routed_e) are structurally zero.
Programs are cached per Ca (multiple of 128), so any input works.

W1 is host-packed into (F/128, 128, D/128, 128) blocks so each stationary
tile streams as contiguous 4KB runs per partition (512B runs measured at
~50GB/s vs ~contiguous at full rate).

b1/b2 are structurally zero in this problem (setup_inputs fills zeros); a
host-side fallback handles nonzero b2, and nonzero b1 is unsupported.
"""

import math
import sys

import numpy as np

for _p in ("/opt/trn_rl_repo",):
    if _p not in sys.path:
        sys.path.append(_p)

# Problem dims (hardcoded per contract)
T, E, D, F, C, K = 4096, 8, 1024, 4096, 1536, 2
N_CORES = 8
P = 128
G = 4  # F-dim groups for the fused hT staging
KO = D // P  # 8 k-subtiles of the D contraction
NF = F // P  # 32 f-chunks of 128
FPG = NF // G  # f-chunks per group

_PROGRAMS = {}  # c_act -> (nc, names)


def _c_chunks(c_act):
    """Split c_act into matmul free-dim chunks <= 512, preferring >= 256
    (float32r streams at 1 cyc/row only for N >= 256)."""
    chunks = []
    rem = c_act
    while rem > 0:
        if rem > 512:
            if rem - 512 >= 256 or rem == 1024:
                take = 512
            else:  # rem in (512, 768): split evenly-ish to keep both >= 256
                take = 384
        else:
            take = rem
        chunks.append(take)
        rem -= take
    return chunks


def _build_program(c_act):
    import concourse.mybir as mybir
    import concourse.tile as tile
    from concourse import bacc

    f32 = mybir.dt.float32
    f32r = mybir.dt.float32r
    Relu = mybir.ActivationFunctionType.Relu

    CS = c_act // P  # c-subtiles for MM2
    ND = D // 512  # 2 n-chunks of 512 for MM2
    chunks = _c_chunks(c_act)

    nc = bacc.Bacc(None, target_bir_lowering=False, debug=False)

    with tile.TileContext(nc) as tc:
        with tc.tile_pool(name="dram", bufs=1, space="DRAM") as dram:
            # w1 block-packed on host: (NF, P, KO, P); [fg] -> [ki, ko, f] tile
            w1 = dram.tile((NF, P, KO, P), f32r, kind="ExternalInput", name="w1")
            w2 = dram.tile((F, D), f32r, kind="ExternalInput", name="w2")
            xeT = dram.tile((D, c_act), f32r, kind="ExternalInput", name="xeT")
            y = dram.tile((c_act, D), f32, kind="ExternalOutput", name="y")

        xeT_r = xeT[:].rearrange("(ko ki) c -> ki ko c", ki=P)

        with (
            tc.tile_pool(name="const", bufs=1) as constp,
            tc.tile_pool(name="xe", bufs=1) as xep,
            tc.tile_pool(name="ht", bufs=1) as htp,
            tc.tile_pool(name="ysb", bufs=1) as yp,
            tc.tile_pool(name="w1t", bufs=4) as w1p,
            # SBUF/partition ~= 96*c_act B + w2 bufs*32KB + ~20KB; cap 208KB
            tc.tile_pool(name="w2t", bufs=2 if c_act <= 1280 else 1) as w2p,
            tc.tile_pool(name="ps", bufs=2, space="PSUM") as psp,
        ):
            zero = constp.tile([P, 1], f32)
            nc.any.memset(zero[:], 0.0)

            # first stationary tile ahead of everything: it heads its DMA
            # queue so the PE can start ~10us in instead of ~25us
            w1_first = w1p.tile([P, KO, P], f32r, name="w1_t")
            nc.sync.dma_start(w1_first[:], w1[0])

            # xe split per (ko, chunk) in consumption order: small transfers
            # land progressively so MM1(fc0) streams as they arrive
            # alternate issuing engine: scalar's HWDGE queue set is free at
            # kernel start (first relu evict is ~16us in), doubling early
            # DMA parallelism for the PE-critical xe stream
            xe_sb = xep.tile([P, KO, c_act], f32r)
            n_xe = 0
            for ko in range(KO):
                c0 = 0
                for cw in chunks:
                    eng = nc.sync if n_xe % 2 == 0 else nc.scalar
                    eng.dma_start(
                        xe_sb[:, ko, c0 : c0 + cw], xeT_r[:, ko, c0 : c0 + cw]
                    )
                    n_xe += 1
                    c0 += cw

            y_sb = yp.tile([P, CS, D], f32)
            hT = htp.tile([P, FPG, c_act], f32r)

            # chunk index -> (c offset, width)
            offs = []
            c0 = 0
            for cw in chunks:
                offs.append((c0, cw))
                c0 += cw

            def mm1_sweep(g, idxs, use_first):
                """One fc-sweep of MM1 over the given c-chunk indices."""
                for fc in range(FPG):
                    fg = g * FPG + fc
                    if use_first and fc == 0:
                        w1_t = w1_first
                    else:
                        w1_t = w1p.tile([P, KO, P], f32r, name="w1_t")
                        nc.sync.dma_start(w1_t[:], w1[fg])
                    ph = {
                        i: psp.tile([P, chunks[i]], f32, name=f"p{i}", tag=f"p{i}")
                        for i in idxs
                    }
                    for k in range(KO):
                        for i in idxs:
                            c0, cw = offs[i]
                            nc.tensor.matmul(
                                ph[i][:],
                                w1_t[:, k, :],
                                xe_sb[:, k, c0 : c0 + cw],
                                start=(k == 0),
                                stop=(k == KO - 1),
                            )
                    for i in idxs:
                        c0, cw = offs[i]
                        nc.scalar.activation(
                            hT[:, fc, c0 : c0 + cw], ph[i][:], Relu, bias=zero[:]
                        )

            for g in range(G):
                # ---- MM1: hT[group] = relu(W1[:, group].T @ xeT) ----
                mm1_sweep(g, list(range(len(chunks))), use_first=(g == 0))

                # W2 tiles for this group (emitted after MM1 so the per-queue
                # DMA FIFOs serve the w1/xe tiles PE needs first)
                w2_t = w2p.tile([P, FPG, D], f32r, name="w2_t")
                for fs in range(FPG):
                    fg = g * FPG + fs
                    nc.sync.dma_start(w2_t[:, fs, :], w2[fg * P : (fg + 1) * P, :])

                # ---- MM2: y[group contribution] = hT.T @ W2[group] ----
                for cs in range(CS):
                    py = [
                        psp.tile([P, 512], f32, name=f"py{dh}", tag=f"p{dh}")
                        for dh in range(ND)
                    ]
                    for fs in range(FPG):
                        for dh in range(ND):
                            nc.tensor.matmul(
                                py[dh][:],
                                hT[:, fs, cs * P : (cs + 1) * P],
                                w2_t[:, fs, dh * 512 : (dh + 1) * 512],
                                start=(fs == 0),
                                stop=(fs == FPG - 1),
                            )
                    for dh in range(ND):
                        dst = y_sb[:, cs, dh * 512 : (dh + 1) * 512]
                        if g == 0:
                            nc.vector.tensor_copy(dst, py[dh][:])
                        else:
                            nc.vector.tensor_add(dst, dst, py[dh][:])
                        if g == G - 1:
                            nc.sync.dma_start(
                                y[cs * P : (cs + 1) * P, dh * 512 : (dh + 1) * 512],
                                dst,
                            )

    nc.compile()
    names = dict(w1=w1.name, w2=w2.name, xeT=xeT.name, y=y.name)
    return nc, names


def _get_program(c_act):
    if c_act not in _PROGRAMS:
        _PROGRAMS[c_act] = _build_program(c_act)
    return _PROGRAMS[c_act]


# test.py can set RUN_KWARGS (e.g. dict(trace=True)) and read LAST_RESULTS
RUN_KWARGS = {}
LAST_RESULTS = None


def kernel(x, route_mask, route_weight, W1, b1, W2, b2):
    from concourse.bass_utils import run_bass_kernel_spmd

    global LAST_RESULTS

    x = np.asarray(x, dtype=np.float32)
    route_mask = np.asarray(route_mask, dtype=bool)
    route_weight = np.asarray(route_weight, dtype=np.float32)
    W1 = np.asarray(W1, dtype=np.float32)
    W2 = np.asarray(W2, dtype=np.float32)
    b1 = np.asarray(b1, dtype=np.float32)
    b2 = np.asarray(b2, dtype=np.float32)
    if np.any(b1):
        raise NotImplementedError("nonzero b1 not supported")

    # --- routing: per-expert top-C tokens by route weight (ties -> lower idx) ---
    w_et = np.where(route_mask.T, route_weight.T, -np.inf)  # (E, T)
    order = np.argsort(-w_et, axis=1, kind="stable")[:, :C]  # (E, C) token ids
    vals = np.take_along_axis(w_et, order, axis=1)  # (E, C)
    valid = np.isfinite(vals)  # (E, C)
    gain = np.where(valid, vals, 0.0).astype(np.float32)  # (E, C)

    # active capacity: valid slots are a prefix (sorted by weight desc)
    n_e = valid.sum(axis=1)
    c_act = min(C, int(math.ceil(max(1, n_e.max()) / P)) * P)

    nc, names = _get_program(c_act)

    # --- dispatch: gather + fold gain, per expert ---
    in_maps = []
    for e in range(E):
        xe = x[order[e, :c_act]] * gain[e, :c_act][:, None]  # (Ca, D)
        xeT_np = np.ascontiguousarray(xe.T)  # (D, Ca)
        w1b = np.ascontiguousarray(
            W1[e].reshape(KO, P, NF, P).transpose(2, 1, 0, 3)
        )  # (NF, P, KO, P)
        in_maps.append({names["w1"]: w1b, names["xeT"]: xeT_np, names["w2"]: W2[e]})

    res = run_bass_kernel_spmd(nc, in_maps, list(range(N_CORES)), **RUN_KWARGS)
    LAST_RESULTS = res

    # --- combine: scatter-add per-expert outputs ---
    y = np.zeros((T, D), np.float32)
    for e in range(E):
        ye = res.results[e][names["y"]]  # (Ca, D)
        m = valid[e, :c_act]
        if np.any(b2):
            ye = ye + gain[e, :c_act][:, None] * b2[e][None, :]
        y[order[e, :c_act][m]] += ye[m]
    return y


# revision 15
# speedup vs baseline: 1.1785x; 1.1785x over previous
"""MoE (top-K routing, per-expert capacity) Trainium2 kernel.

Strategy: expert parallelism across 8 NeuronCores (E=8, one expert per core).
 - Host: routing top-C selection per expert (tiny: E x T scores), gather of
   dispatched tokens, and fold of the combine weights ("gain") into the
   dispatched activations. gain >= 0 (softmax outputs), so
   gain * (relu(xe@W1)@W2) == relu((gain*xe)@W1)@W2 exactly in math terms.
 - Device (per core): fused 2-layer MLP in a single hand-written Tile kernel:
       hT = relu(W1.T @ xeT)   (F, Ca)   hT kept in SBUF, F in G groups
       y  = hT.T @ W2          (Ca, D)   PSUM-accumulated per group,
                                         DVE-accumulated across groups
   float32r matmuls (full PE stream rate, fp32-class data).
 - Host: per-expert scatter-add of y_e back into the (T, D) output.

Only the active capacity prefix Ca <= C is computed: top-C ordering sorts
valid slots first, so slots >= max_e(n_routed_e) are structurally zero.
Programs are cached per Ca (multiple of 128), so any input works.

W1 is host-packed into (F/128, 128, D/128, 128) blocks so each stationary
tile streams as contiguous 4KB runs per partition (512B runs measured at
~50GB/s vs ~contiguous at full rate).

b1/b2 are structurally zero in this problem (setup_inputs fills zeros); a
host-side fallback handles nonzero b2, and nonzero b1 is unsupported.
"""

import math
import sys

import numpy as np

for _p in ("/opt/trn_rl_repo",):
    if _p not in sys.path:
        sys.path.append(_p)

# Problem dims (hardcoded per contract)
T, E, D, F, C, K = 4096, 8, 1024, 4096, 1536, 2
N_CORES = 8
P = 128
G = 4  # F-dim groups for the fused hT staging
KO = D // P  # 8 k-subtiles of the D contraction
NF = F // P  # 32 f-chunks of 128
FPG = NF // G  # f-chunks per group

_PROGRAMS = {}  # c_act -> (nc, names)


def _c_chunks(c_act):
    """Split c_act into matmul free-dim chunks <= 512, preferring >= 256
    (float32r streams at 1 cyc/row only for N >= 256)."""
    chunks = []
    rem = c_act
    while rem > 0:
        if rem > 512:
            if rem - 512 >= 256 or rem == 1024:
                take = 512
            else:  # rem in (512, 768): split evenly-ish to keep both >= 256
                take = 384
        else:
            take = rem
        chunks.append(take)
        rem -= take
    return chunks


def _build_program(c_act):
    import concourse.mybir as mybir
    import concourse.tile as tile
    from concourse import bacc

    f32 = mybir.dt.float32
    f32r = mybir.dt.float32r
    Relu = mybir.ActivationFunctionType.Relu

    CS = c_act // P  # c-subtiles for MM2
    ND = D // 512  # 2 n-chunks of 512 for MM2
    chunks = _c_chunks(c_act)

    nc = bacc.Bacc(None, target_bir_lowering=False, debug=False)

    with tile.TileContext(nc) as tc:
        with tc.tile_pool(name="dram", bufs=1, space="DRAM") as dram:
            # w1 block-packed on host: (NF, P, KO, P); [fg] -> [ki, ko, f] tile
            w1 = dram.tile((NF, P, KO, P), f32r, kind="ExternalInput", name="w1")
            w2 = dram.tile((F, D), f32r, kind="ExternalInput", name="w2")
            xeT = dram.tile((D, c_act), f32r, kind="ExternalInput", name="xeT")
            y = dram.tile((c_act, D), f32, kind="ExternalOutput", name="y")

        xeT_r = xeT[:].rearrange("(ko ki) c -> ki ko c", ki=P)

        with (
            tc.tile_pool(name="const", bufs=1) as constp,
            tc.tile_pool(name="xe", bufs=1) as xep,
            tc.tile_pool(name="ht", bufs=1) as htp,
            tc.tile_pool(name="ysb", bufs=1) as yp,
            tc.tile_pool(name="w1t", bufs=4) as w1p,
            # SBUF/partition ~= 96*c_act B + w2 bufs*32KB + ~20KB; cap 208KB
            tc.tile_pool(name="w2t", bufs=2 if c_act <= 1280 else 1) as w2p,
            tc.tile_pool(name="ps", bufs=2, space="PSUM") as psp,
        ):
            zero = constp.tile([P, 1], f32)
            nc.any.memset(zero[:], 0.0)

            # first stationary tile ahead of everything: it heads its DMA
            # queue so the PE can start ~10us in instead of ~25us
            w1_first = w1p.tile([P, KO, P], f32r, name="w1_t")
            nc.sync.dma_start(w1_first[:], w1[0])

            # xe split per (ko, chunk) in consumption order: small transfers
            # land progressively so MM1(fc0) streams as they arrive
            xe_sb = xep.tile([P, KO, c_act], f32r)
            for ko in range(KO):
                c0 = 0
                for cw in chunks:
                    nc.sync.dma_start(
                        xe_sb[:, ko, c0 : c0 + cw], xeT_r[:, ko, c0 : c0 + cw]
                    )
                    c0 += cw

            y_sb = yp.tile([P, CS, D], f32)
            hT = htp.tile([P, FPG, c_act], f32r)

            # chunk index -> (c offset, width)
            offs = []
            c0 = 0
            for cw in chunks:
                offs.append((c0, cw))
                c0 += cw

            def mm1_sweep(g, idxs, use_first):
                """One fc-sweep of MM1 over the given c-chunk indices."""
                for fc in range(FPG):
                    fg = g * FPG + fc
                    if use_first and fc == 0:
                        w1_t = w1_first
                    else:
                        w1_t = w1p.tile([P, KO, P], f32r, name="w1_t")
                        nc.sync.dma_start(w1_t[:], w1[fg])
                    ph = {
                        i: psp.tile([P, chunks[i]], f32, name=f"p{i}", tag=f"p{i}")
                        for i in idxs
                    }
                    for k in range(KO):
                        for i in idxs:
                            c0, cw = offs[i]
                            nc.tensor.matmul(
                                ph[i][:],
                                w1_t[:, k, :],
                                xe_sb[:, k, c0 : c0 + cw],
                                start=(k == 0),
                                stop=(k == KO - 1),
                            )
                    for i in idxs:
                        c0, cw = offs[i]
                        nc.scalar.activation(
                            hT[:, fc, c0 : c0 + cw], ph[i][:], Relu, bias=zero[:]
                        )

            for g in range(G):
                # ---- MM1: hT[group] = relu(W1[:, group].T @ xeT) ----
                mm1_sweep(g, list(range(len(chunks))), use_first=(g == 0))

                # W2 tiles for this group (emitted after MM1 so the per-queue
                # DMA FIFOs serve the w1/xe tiles PE needs first)
                w2_t = w2p.tile([P, FPG, D], f32r, name="w2_t")
                for fs in range(FPG):
                    fg = g * FPG + fs
                    nc.sync.dma_start(w2_t[:, fs, :], w2[fg * P : (fg + 1) * P, :])

                # ---- MM2: y[group contribution] = hT.T @ W2[group] ----
                for cs in range(CS):
                    py = [
                        psp.tile([P, 512], f32, name=f"py{dh}", tag=f"p{dh}")
                        for dh in range(ND)
                    ]
                    for fs in range(FPG):
                        for dh in range(ND):
                            nc.tensor.matmul(
                                py[dh][:],
                                hT[:, fs, cs * P : (cs + 1) * P],
                                w2_t[:, fs, dh * 512 : (dh + 1) * 512],
                                start=(fs == 0),
                                stop=(fs == FPG - 1),
                            )
                    for dh in range(ND):
                        dst = y_sb[:, cs, dh * 512 : (dh + 1) * 512]
                        if g == 0:
                            nc.vector.tensor_copy(dst, py[dh][:])
                        else:
                            nc.vector.tensor_add(dst, dst, py[dh][:])
                        if g == G - 1:
                            nc.sync.dma_start(
                                y[cs * P : (cs + 1) * P, dh * 512 : (dh + 1) * 512],
                                dst,
                            )

    nc.compile()
    names = dict(w1=w1.name, w2=w2.name, xeT=xeT.name, y=y.name)
    return nc, names


def _get_program(c_act):
    if c_act not in _PROGRAMS:
        _PROGRAMS[c_act] = _build_program(c_act)
    return _PROGRAMS[c_act]


# test.py can set RUN_KWARGS (e.g. dict(trace=True)) and read LAST_RESULTS
RUN_KWARGS = {}
LAST_RESULTS = None


def kernel(x, route_mask, route_weight, W1, b1, W2, b2):
    from concourse.bass_utils import run_bass_kernel_spmd

    global LAST_RESULTS

    x = np.asarray(x, dtype=np.float32)
    route_mask = np.asarray(route_mask, dtype=bool)
    route_weight = np.asarray(route_weight, dtype=np.float32)
    W1 = np.asarray(W1, dtype=np.float32)
    W2 = np.asarray(W2, dtype=np.float32)
    b1 = np.asarray(b1, dtype=np.float32)
    b2 = np.asarray(b2, dtype=np.float32)
    if np.any(b1):
        raise NotImplementedError("nonzero b1 not supported")

    # --- routing: per-expert top-C tokens by route weight (ties -> lower idx) ---
    w_et = np.where(route_mask.T, route_weight.T, -np.inf)  # (E, T)
    order = np.argsort(-w_et, axis=1, kind="stable")[:, :C]  # (E, C) token ids
    vals = np.take_along_axis(w_et, order, axis=1)  # (E, C)
    valid = np.isfinite(vals)  # (E, C)
    gain = np.where(valid, vals, 0.0).astype(np.float32)  # (E, C)

    # active capacity: valid slots are a prefix (sorted by weight desc)
    n_e = valid.sum(axis=1)
    c_act = min(C, int(math.ceil(max(1, n_e.max()) / P)) * P)

    nc, names = _get_program(c_act)

    # --- dispatch: gather + fold gain, per expert ---
    in_maps = []
    for e in range(E):
        xe = x[order[e, :c_act]] * gain[e, :c_act][:, None]  # (Ca, D)
        xeT_np = np.ascontiguousarray(xe.T)  # (D, Ca)
        w1b = np.ascontiguousarray(
            W1[e].reshape(KO, P, NF, P).transpose(2, 1, 0, 3)
        )  # (NF, P, KO, P)
        in_maps.append({names["w1"]: w1b, names["xeT"]: xeT_np, names["w2"]: W2[e]})

    res = run_bass_kernel_spmd(nc, in_maps, list(range(N_CORES)), **RUN_KWARGS)
    LAST_RESULTS = res

    # --- combine: scatter-add per-expert outputs ---
    y = np.zeros((T, D), np.float32)
    for e in range(E):
        ye = res.results[e][names["y"]]  # (Ca, D)
        m = valid[e, :c_act]
        if np.any(b2):
            ye = ye + gain[e, :c_act][:, None] * b2[e][None, :]
        y[order[e, :c_act][m]] += ye[m]
    return y


# revision 16
# speedup vs baseline: 1.1866x; 1.0069x over previous
"""MoE (top-K routing, per-expert capacity) Trainium2 kernel.

Strategy: expert parallelism across 8 NeuronCores (E=8, one expert per core).
 - Host: routing top-C selection per expert (tiny: E x T scores), gather of
   dispatched tokens, and fold of the combine weights ("gain") into the
   dispatched activations. gain >= 0 (softmax outputs), so
   gain * (relu(xe@W1)@W2) == relu((gain*xe)@W1)@W2 exactly in math terms.
 - Device (per core): fused 2-layer MLP in a single hand-written Tile kernel:
       hT = relu(W1.T @ xeT)   (F, Ca)   hT kept in SBUF, F in G groups
       y  = hT.T @ W2          (Ca, D)   PSUM-accumulated per group,
                                         DVE-accumulated across groups
   float32r matmuls (full PE stream rate, fp32-class data).
 - Host: per-expert scatter-add of y_e back into the (T, D) output.

Only the active capacity prefix Ca <= C is computed: top-C ordering sorts
valid slots first, so slots >= max_e(n_routed_e) are structurally zero.
Programs are cached per Ca (multiple of 128), so any input works.

W1 is host-packed into (F/128, 128, D/128, 128) blocks so each stationary
tile streams as contiguous 4KB runs per partition (512B runs measured at
~50GB/s vs ~contiguous at full rate).

b1/b2 are structurally zero in this problem (setup_inputs fills zeros); a
host-side fallback handles nonzero b2, and nonzero b1 is unsupported.
"""

import math
import sys

import numpy as np

for _p in ("/opt/trn_rl_repo",):
    if _p not in sys.path:
        sys.path.append(_p)

# Problem dims (hardcoded per contract)
T, E, D, F, C, K = 4096, 8, 1024, 4096, 1536, 2
N_CORES = 8
P = 128
G = 4  # F-dim groups for the fused hT staging
KO = D // P  # 8 k-subtiles of the D contraction
NF = F // P  # 32 f-chunks of 128
FPG = NF // G  # f-chunks per group

_PROGRAMS = {}  # c_act -> (nc, names)


def _c_chunks(c_act):
    """Split c_act into matmul free-dim chunks <= 512, preferring >= 256
    (float32r streams at 1 cyc/row only for N >= 256)."""
    chunks = []
    rem = c_act
    while rem > 0:
        if rem > 512:
            if rem - 512 >= 256 or rem == 1024:
                take = 512
            else:  # rem in (512, 768): split evenly-ish to keep both >= 256
                take = 384
        else:
            take = rem
        chunks.append(take)
        rem -= take
    return chunks


def _build_program(c_act):
    import concourse.mybir as mybir
    import concourse.tile as tile
    from concourse import bacc

    f32 = mybir.dt.float32
    f32r = mybir.dt.float32r
    Relu = mybir.ActivationFunctionType.Relu

    CS = c_act // P  # c-subtiles for MM2
    ND = D // 512  # 2 n-chunks of 512 for MM2
    chunks = _c_chunks(c_act)

    nc = bacc.Bacc(None, target_bir_lowering=False, debug=False)

    with tile.TileContext(nc) as tc:
        with tc.tile_pool(name="dram", bufs=1, space="DRAM") as dram:
            # w1 block-packed on host: (NF, P, KO, P); [fg] -> [ki, ko, f] tile
            w1 = dram.tile((NF, P, KO, P), f32r, kind="ExternalInput", name="w1")
            w2 = dram.tile((F, D), f32r, kind="ExternalInput", name="w2")
            xeT = dram.tile((D, c_act), f32r, kind="ExternalInput", name="xeT")
            y = dram.tile((c_act, D), f32, kind="ExternalOutput", name="y")

        xeT_r = xeT[:].rearrange("(ko ki) c -> ki ko c", ki=P)

        with (
            tc.tile_pool(name="const", bufs=1) as constp,
            tc.tile_pool(name="xe", bufs=1) as xep,
            tc.tile_pool(name="ht", bufs=1) as htp,
            tc.tile_pool(name="ysb", bufs=1) as yp,
            tc.tile_pool(name="w1t", bufs=4) as w1p,
            # SBUF/partition ~= 96*c_act B + w2 bufs*32KB + ~20KB; cap 208KB
            tc.tile_pool(name="w2t", bufs=2 if c_act <= 1280 else 1) as w2p,
            tc.tile_pool(name="ps", bufs=2, space="PSUM") as psp,
        ):
            zero = constp.tile([P, 1], f32)
            nc.any.memset(zero[:], 0.0)

            # first stationary tile ahead of everything: it heads its DMA
            # queue so the PE can start ~10us in instead of ~25us
            w1_first = w1p.tile([P, KO, P], f32r, name="w1_t")
            nc.sync.dma_start(w1_first[:], w1[0])

            # xe split per (ko, chunk) in consumption order: small transfers
            # land progressively so MM1(fc0) streams as they arrive
            xe_sb = xep.tile([P, KO, c_act], f32r)
            for ko in range(KO):
                c0 = 0
                for cw in chunks:
                    nc.sync.dma_start(
                        xe_sb[:, ko, c0 : c0 + cw], xeT_r[:, ko, c0 : c0 + cw]
                    )
                    c0 += cw

            y_sb = yp.tile([P, CS, D], f32)
            hT = htp.tile([P, FPG, c_act], f32r)

            # chunk index -> (c offset, width)
            offs = []
            c0 = 0
            for cw in chunks:
                offs.append((c0, cw))
                c0 += cw

            def mm1_sweep(g, idxs, use_first):
                """One fc-sweep of MM1 over the given c-chunk indices."""
                for fc in range(FPG):
                    fg = g * FPG + fc
                    if use_first and fc == 0:
                        w1_t = w1_first
                    else:
                        w1_t = w1p.tile([P, KO, P], f32r, name="w1_t")
                        nc.sync.dma_start(w1_t[:], w1[fg])
                    ph = {
                        i: psp.tile([P, chunks[i]], f32, name=f"p{i}", tag=f"p{i}")
                        for i in idxs
                    }
                    for k in range(KO):
                        # smallest chunk first: the trailing wide matmul
                        # hides the next k-step's LDWEIGHTS
                        for i in sorted(idxs, key=lambda j: chunks[j]):
                            c0, cw = offs[i]
                            nc.tensor.matmul(
                                ph[i][:],
                                w1_t[:, k, :],
                                xe_sb[:, k, c0 : c0 + cw],
                                start=(k == 0),
                                stop=(k == KO - 1),
                            )
                    for i in idxs:
                        c0, cw = offs[i]
                        nc.scalar.activation(
                            hT[:, fc, c0 : c0 + cw], ph[i][:], Relu, bias=zero[:]
                        )

            for g in range(G):
                # ---- MM1: hT[group] = relu(W1[:, group].T @ xeT) ----
                mm1_sweep(g, list(range(len(chunks))), use_first=(g == 0))

                # W2 tiles for this group (emitted after MM1 so the per-queue
                # DMA FIFOs serve the w1/xe tiles PE needs first)
                w2_t = w2p.tile([P, FPG, D], f32r, name="w2_t")
                for fs in range(FPG):
                    fg = g * FPG + fs
                    nc.sync.dma_start(w2_t[:, fs, :], w2[fg * P : (fg + 1) * P, :])

                # ---- MM2: y[group contribution] = hT.T @ W2[group] ----
                for cs in range(CS):
                    py = [
                        psp.tile([P, 512], f32, name=f"py{dh}", tag=f"p{dh}")
                        for dh in range(ND)
                    ]
                    for fs in range(FPG):
                        for dh in range(ND):
                            nc.tensor.matmul(
                                py[dh][:],
                                hT[:, fs, cs * P : (cs + 1) * P],
                                w2_t[:, fs, dh * 512 : (dh + 1) * 512],
                                start=(fs == 0),
                                stop=(fs == FPG - 1),
                            )
                    for dh in range(ND):
                        dst = y_sb[:, cs, dh * 512 : (dh + 1) * 512]
                        if g == 0:
                            nc.vector.tensor_copy(dst, py[dh][:])
                        else:
                            nc.vector.tensor_add(dst, dst, py[dh][:])
                        if g == G - 1:
                            nc.sync.dma_start(
                                y[cs * P : (cs + 1) * P, dh * 512 : (dh + 1) * 512],
                                dst,
                            )

    nc.compile()
    names = dict(w1=w1.name, w2=w2.name, xeT=xeT.name, y=y.name)
    return nc, names


def _get_program(c_act):
    if c_act not in _PROGRAMS:
        _PROGRAMS[c_act] = _build_program(c_act)
    return _PROGRAMS[c_act]


# test.py can set RUN_KWARGS (e.g. dict(trace=True)) and read LAST_RESULTS
RUN_KWARGS = {}
LAST_RESULTS = None


def kernel(x, route_mask, route_weight, W1, b1, W2, b2):
    from concourse.bass_utils import run_bass_kernel_spmd

    global LAST_RESULTS

    x = np.asarray(x, dtype=np.float32)
    route_mask = np.asarray(route_mask, dtype=bool)
    route_weight = np.asarray(route_weight, dtype=np.float32)
    W1 = np.asarray(W1, dtype=np.float32)
    W2 = np.asarray(W2, dtype=np.float32)
    b1 = np.asarray(b1, dtype=np.float32)
    b2 = np.asarray(b2, dtype=np.float32)
    if np.any(b1):
        raise NotImplementedError("nonzero b1 not supported")

    # --- routing: per-expert top-C tokens by route weight (ties -> lower idx) ---
    w_et = np.where(route_mask.T, route_weight.T, -np.inf)  # (E, T)
    order = np.argsort(-w_et, axis=1, kind="stable")[:, :C]  # (E, C) token ids
    vals = np.take_along_axis(w_et, order, axis=1)  # (E, C)
    valid = np.isfinite(vals)  # (E, C)
    gain = np.where(valid, vals, 0.0).astype(np.float32)  # (E, C)

    # active capacity: valid slots are a prefix (sorted by weight desc)
    n_e = valid.sum(axis=1)
    c_act = min(C, int(math.ceil(max(1, n_e.max()) / P)) * P)

    nc, names = _get_program(c_act)

    # --- dispatch: gather + fold gain, per expert ---
    in_maps = []
    for e in range(E):
        xe = x[order[e, :c_act]] * gain[e, :c_act][:, None]  # (Ca, D)
        xeT_np = np.ascontiguousarray(xe.T)  # (D, Ca)
        w1b = np.ascontiguousarray(
            W1[e].reshape(KO, P, NF, P).transpose(2, 1, 0, 3)
        )  # (NF, P, KO, P)
        in_maps.append({names["w1"]: w1b, names["xeT"]: xeT_np, names["w2"]: W2[e]})

    res = run_bass_kernel_spmd(nc, in_maps, list(range(N_CORES)), **RUN_KWARGS)
    LAST_RESULTS = res

    # --- combine: scatter-add per-expert outputs ---
    y = np.zeros((T, D), np.float32)
    for e in range(E):
        ye = res.results[e][names["y"]]  # (Ca, D)
        m = valid[e, :c_act]
        if np.any(b2):
            ye = ye + gain[e, :c_act][:, None] * b2[e][None, :]
        y[order[e, :c_act][m]] += ye[m]
    return y


# revision 17
# speedup vs baseline: 1.1871x; 1.0004x over previous
"""MoE (top-K routing, per-expert capacity) Trainium2 kernel.

Strategy: expert parallelism across 8 NeuronCores (E=8, one expert per core).
 - Host: routing top-C selection per expert (tiny: E x T scores), gather of
   dispatched tokens, and fold of the combine weights ("gain") into the
   dispatched activations. gain >= 0 (softmax outputs), so
   gain * (relu(xe@W1)@W2) == relu((gain*xe)@W1)@W2 exactly in math terms.
 - Device (per core): fused 2-layer MLP in a single hand-written Tile kernel:
       hT = relu(W1.T @ xeT)   (F, Ca)   hT kept in SBUF, F in G groups
       y  = hT.T @ W2          (Ca, D)   PSUM-accumulated per group,
                                         DVE-accumulated across groups
   float32r matmuls (full PE stream rate, fp32-class data).
 - Host: per-expert scatter-add of y_e back into the (T, D) output.

Only the active capacity prefix Ca <= C is computed: top-C ordering sorts
valid slots first, so slots >= max_e(n_routed_e) are structurally zero.
Programs are cached per Ca (multiple of 128), so any input works.

W1 is host-packed into (F/128, 128, D/128, 128) blocks so each stationary
tile streams as contiguous 4KB runs per partition (512B runs measured at
~50GB/s vs ~contiguous at full rate).

b1/b2 are structurally zero in this problem (setup_inputs fills zeros); a
host-side fallback handles nonzero b2, and nonzero b1 is unsupported.
"""

import math
import sys

import numpy as np

for _p in ("/opt/trn_rl_repo",):
    if _p not in sys.path:
        sys.path.append(_p)

# Problem dims (hardcoded per contract)
T, E, D, F, C, K = 4096, 8, 1024, 4096, 1536, 2
N_CORES = 8
P = 128
G = 4  # F-dim groups for the fused hT staging
KO = D // P  # 8 k-subtiles of the D contraction
NF = F // P  # 32 f-chunks of 128
FPG = NF // G  # f-chunks per group

_PROGRAMS = {}  # c_act -> (nc, names)


def _c_chunks(c_act):
    """Split c_act into matmul free-dim chunks <= 512, preferring >= 256
    (float32r streams at 1 cyc/row only for N >= 256)."""
    chunks = []
    rem = c_act
    while rem > 0:
        if rem > 512:
            if rem - 512 >= 256 or rem == 1024:
                take = 512
            else:  # rem in (512, 768): split evenly-ish to keep both >= 256
                take = 384
        else:
            take = rem
        chunks.append(take)
        rem -= take
    return chunks


def _build_program(c_act):
    import concourse.mybir as mybir
    import concourse.tile as tile
    from concourse import bacc

    f32 = mybir.dt.float32
    f32r = mybir.dt.float32r
    Relu = mybir.ActivationFunctionType.Relu

    CS = c_act // P  # c-subtiles for MM2
    ND = D // 512  # 2 n-chunks of 512 for MM2
    chunks = _c_chunks(c_act)

    nc = bacc.Bacc(None, target_bir_lowering=False, debug=False)

    with tile.TileContext(nc) as tc:
        with tc.tile_pool(name="dram", bufs=1, space="DRAM") as dram:
            # w1 block-packed on host: (NF, P, KO, P); [fg] -> [ki, ko, f] tile
            w1 = dram.tile((NF, P, KO, P), f32r, kind="ExternalInput", name="w1")
            w2 = dram.tile((F, D), f32r, kind="ExternalInput", name="w2")
            xeT = dram.tile((D, c_act), f32r, kind="ExternalInput", name="xeT")
            y = dram.tile((c_act, D), f32, kind="ExternalOutput", name="y")

        xeT_r = xeT[:].rearrange("(ko ki) c -> ki ko c", ki=P)

        with (
            tc.tile_pool(name="const", bufs=1) as constp,
            tc.tile_pool(name="xe", bufs=1) as xep,
            tc.tile_pool(name="ht", bufs=1) as htp,
            tc.tile_pool(name="ysb", bufs=1) as yp,
            tc.tile_pool(name="w1t", bufs=6) as w1p,
            # SBUF/partition ~= 96*c_act B + w2 bufs*32KB + ~20KB; cap 208KB
            tc.tile_pool(name="w2t", bufs=2 if c_act <= 1280 else 1) as w2p,
            tc.tile_pool(name="ps", bufs=2, space="PSUM") as psp,
        ):
            zero = constp.tile([P, 1], f32)
            nc.any.memset(zero[:], 0.0)

            # first stationary tile ahead of everything: it heads its DMA
            # queue so the PE can start ~10us in instead of ~25us
            w1_first = w1p.tile([P, KO, P], f32r, name="w1_t")
            nc.sync.dma_start(w1_first[:], w1[0])

            # xe split per (ko, chunk) in consumption order: small transfers
            # land progressively so MM1(fc0) streams as they arrive
            xe_sb = xep.tile([P, KO, c_act], f32r)
            for ko in range(KO):
                c0 = 0
                for cw in chunks:
                    nc.sync.dma_start(
                        xe_sb[:, ko, c0 : c0 + cw], xeT_r[:, ko, c0 : c0 + cw]
                    )
                    c0 += cw

            y_sb = yp.tile([P, CS, D], f32)
            hT = htp.tile([P, FPG, c_act], f32r)

            # chunk index -> (c offset, width)
            offs = []
            c0 = 0
            for cw in chunks:
                offs.append((c0, cw))
                c0 += cw

            def mm1_sweep(g, idxs, use_first):
                """One fc-sweep of MM1 over the given c-chunk indices."""
                for fc in range(FPG):
                    fg = g * FPG + fc
                    if use_first and fc == 0:
                        w1_t = w1_first
                    else:
                        w1_t = w1p.tile([P, KO, P], f32r, name="w1_t")
                        nc.sync.dma_start(w1_t[:], w1[fg])
                    ph = {
                        i: psp.tile([P, chunks[i]], f32, name=f"p{i}", tag=f"p{i}")
                        for i in idxs
                    }
                    for k in range(KO):
                        # smallest chunk first: the trailing wide matmul
                        # hides the next k-step's LDWEIGHTS
                        for i in sorted(idxs, key=lambda j: chunks[j]):
                            c0, cw = offs[i]
                            nc.tensor.matmul(
                                ph[i][:],
                                w1_t[:, k, :],
                                xe_sb[:, k, c0 : c0 + cw],
                                start=(k == 0),
                                stop=(k == KO - 1),
                            )
                    for i in idxs:
                        c0, cw = offs[i]
                        nc.scalar.activation(
                            hT[:, fc, c0 : c0 + cw], ph[i][:], Relu, bias=zero[:]
                        )

            for g in range(G):
                # ---- MM1: hT[group] = relu(W1[:, group].T @ xeT) ----
                mm1_sweep(g, list(range(len(chunks))), use_first=(g == 0))

                # W2 tiles for this group (emitted after MM1 so the per-queue
                # DMA FIFOs serve the w1/xe tiles PE needs first)
                w2_t = w2p.tile([P, FPG, D], f32r, name="w2_t")
                for fs in range(FPG):
                    fg = g * FPG + fs
                    nc.sync.dma_start(w2_t[:, fs, :], w2[fg * P : (fg + 1) * P, :])

                # ---- MM2: y[group contribution] = hT.T @ W2[group] ----
                for cs in range(CS):
                    py = [
                        psp.tile([P, 512], f32, name=f"py{dh}", tag=f"p{dh}")
                        for dh in range(ND)
                    ]
                    for fs in range(FPG):
                        for dh in range(ND):
                            nc.tensor.matmul(
                                py[dh][:],
                                hT[:, fs, cs * P : (cs + 1) * P],
                                w2_t[:, fs, dh * 512 : (dh + 1) * 512],
                                start=(fs == 0),
                                stop=(fs == FPG - 1),
                            )
                    for dh in range(ND):
                        dst = y_sb[:, cs, dh * 512 : (dh + 1) * 512]
                        if g == 0:
                            nc.vector.tensor_copy(dst, py[dh][:])
                        else:
                            nc.vector.tensor_add(dst, dst, py[dh][:])
                        if g == G - 1:
                            nc.sync.dma_start(
                                y[cs * P : (cs + 1) * P, dh * 512 : (dh + 1) * 512],
                                dst,
                            )

    nc.compile()
    names = dict(w1=w1.name, w2=w2.name, xeT=xeT.name, y=y.name)
    return nc, names


def _get_program(c_act):
    if c_act not in _PROGRAMS:
        _PROGRAMS[c_act] = _build_program(c_act)
    return _PROGRAMS[c_act]


# test.py can set RUN_KWARGS (e.g. dict(trace=True)) and read LAST_RESULTS
RUN_KWARGS = {}
LAST_RESULTS = None


def kernel(x, route_mask, route_weight, W1, b1, W2, b2):
    from concourse.bass_utils import run_bass_kernel_spmd

    global LAST_RESULTS

    x = np.asarray(x, dtype=np.float32)
    route_mask = np.asarray(route_mask, dtype=bool)
    route_weight = np.asarray(route_weight, dtype=np.float32)
    W1 = np.asarray(W1, dtype=np.float32)
    W2 = np.asarray(W2, dtype=np.float32)
    b1 = np.asarray(b1, dtype=np.float32)
    b2 = np.asarray(b2, dtype=np.float32)
    if np.any(b1):
        raise NotImplementedError("nonzero b1 not supported")

    # --- routing: per-expert top-C tokens by route weight (ties -> lower idx) ---
    w_et = np.where(route_mask.T, route_weight.T, -np.inf)  # (E, T)
    order = np.argsort(-w_et, axis=1, kind="stable")[:, :C]  # (E, C) token ids
    vals = np.take_along_axis(w_et, order, axis=1)  # (E, C)
    valid = np.isfinite(vals)  # (E, C)
    gain = np.where(valid, vals, 0.0).astype(np.float32)  # (E, C)

    # active capacity: valid slots are a prefix (sorted by weight desc)
    n_e = valid.sum(axis=1)
    c_act = min(C, int(math.ceil(max(1, n_e.max()) / P)) * P)

    nc, names = _get_program(c_act)

    # --- dispatch: gather + fold gain, per expert ---
    in_maps = []
    for e in range(E):
        xe = x[order[e, :c_act]] * gain[e, :c_act][:, None]  # (Ca, D)
        xeT_np = np.ascontiguousarray(xe.T)  # (D, Ca)
        w1b = np.ascontiguousarray(
            W1[e].reshape(KO, P, NF, P).transpose(2, 1, 0, 3)
        )  # (NF, P, KO, P)
        in_maps.append({names["w1"]: w1b, names["xeT"]: xeT_np, names["w2"]: W2[e]})

    res = run_bass_kernel_spmd(nc, in_maps, list(range(N_CORES)), **RUN_KWARGS)
    LAST_RESULTS = res

    # --- combine: scatter-add per-expert outputs ---
    y = np.zeros((T, D), np.float32)
    for e in range(E):
        ye = res.results[e][names["y"]]  # (Ca, D)
        m = valid[e, :c_act]
        if np.any(b2):
            ye = ye + gain[e, :c_act][:, None] * b2[e][None, :]
        y[order[e, :c_act][m]] += ye[m]
    return y
